# revision 18
# baseline (speedup 1.0000x reference)
# Trainium2 Bass kernel for the ContextBlock problem.
#
# Reference computation (per sample b):
#   xc    = concat(x0..x3)            [C=1024, HW=4096]
#   attn  = softmax(wm @ xc)          [HW]
#   ctx   = xc @ attn                 [C]
#   mul   = residual-gated MLP stack (sigmoid branch)   [C]
#   add   = residual-gated MLP stack (linear branch)    [C]
#   out   = sum_l (x_l * mul_l + add_l)                 [CL=256, HW]
#
# Distribution: data-parallel over batch, one sample per NeuronCore (B=8).
# No collectives required.
#
# Engine facts this kernel is built around:
#   - PE ramps to 2.4 G cols/s only after ~3us of continuous execution;
#     idle gaps drop it back to 1.2 G. Keep the PE queue dense.
#   - Pool (GpSimd) has NO PSUM access and no TensorScalar/STT opcodes;
#     only tensor_tensor/tensor_copy on SBUF, ~2x slower than DVE.
#   - Scalar (Act) engine does func(scale*in + bias) with per-partition
#     AP scale/bias, reads PSUM, 1.2 GHz.
#   - DVE does STT/tensor_scalar (fp32 scalars), reads PSUM, 0.96 GHz.
#
# Dataflow:
#   phase A (rides the x DMA): PE runs pass1 logit matmuls for all 8
#     slabs interleaved with pass2 pre-transposes of slabs 0-5 so it
#     never idles; Scalar/DVE drain the transpose PSUM tiles to SBUF.
#   softmax: exp on [8,512] rows, sums via tiny matmuls.
#   pass2 ctx: PE contracts the 6 pre-transposed slabs against attn
#     columns; DVE runs scalar_tensor_tensor (x * inv * attn, free-axis
#     accum) for slabs 6-7.
#   gates: branch-pipelined rounds. W1 = v-stationary weight-moving
#     matmuls (mul stream then add stream so the mul branch's
#     copies/transposes/LN overlap the add branch's matmuls); h rows ->
#     columns via [1,128]-stationary 1-col matmuls; LayerNorm small-ops
#     on DVE with Ln/Exp/Relu on Scalar; W2 = hn-stationary; sigmoid =
#     1/(1+exp(-z)).
#   pass3: out rows 0:128 on the PE (diag(mul) matmuls accumulated over
#     levels, bias folded into the PSUM drains), rows 128:256 as
#     x*m+acc chains on Scalar/DVE merged by Pool; per-chunk DMA-out.

import numpy as np
import ml_dtypes
from contextlib import ExitStack

import concourse.bass as bass
import concourse.bacc as bacc
import concourse.mybir as mybir
import concourse.tile as tile

BF = mybir.dt.bfloat16
F32 = mybir.dt.float32
AF = mybir.ActivationFunctionType
ALU = mybir.AluOpType
AX = mybir.AxisListType

B, L, CL, H, W = 8, 4, 256, 64, 64
C = L * CL          # 1024
HW = H * W          # 4096
P = C // 4          # 256
R = 2
EPS = 1e-5
NJ = C // 128       # 8   c-slabs
NT = 6              # slabs transposed for the PE ctx route
NCORES = 8

_CACHE = {}


def _patch_act_tables():
    """Prune our activation functions from every table set except
    natural_log_exp_and_others so the whole kernel runs on ONE set
    (no mid-kernel ACT_TABLE_LOAD switches)."""
    if getattr(bacc, "_act_tables_patched", False):
        return
    from concourse import hw_specs
    orig = hw_specs.get_activation_tables
    mine = {AF.Exp, AF.Ln, AF.Relu, AF.Identity, AF.Copy}
    keep = "natural_log_exp_and_others"

    def patched(arch):
        tabs = orig(arch)
        out = {}
        for name, fns in tabs.items():
            out[name] = set(fns) if name == keep else set(fns) - mine
        return out

    import functools
    patched_cached = functools.cache(patched)
    bacc.get_activation_tables = patched_cached
    bacc._act_tables_patched = True


def _build_nc():
    _patch_act_tables()
    nc = bacc.Bacc()

    x_d = nc.dram_tensor("x", [C, HW], BF, kind="ExternalInput")
    wmc_d = nc.dram_tensor("wmc", [128, 64, 8], BF, kind="ExternalInput")
    bfc_d = nc.dram_tensor("bfc", [128, 1312], BF, kind="ExternalInput")
    sm_d = nc.dram_tensor("smalls", [128, 272], F32, kind="ExternalInput")
    wg1_d = nc.dram_tensor("wg1", [2, 8, 128, 2048], BF, kind="ExternalInput")
    wg2_d = nc.dram_tensor("wg2", [2, 128, 4096], BF, kind="ExternalInput")
    out_d = nc.dram_tensor("out", [CL, HW], F32, kind="ExternalOutput")

    with tile.TileContext(nc) as tc, ExitStack() as ctx:
        resid = ctx.enter_context(tc.tile_pool(name="resid", bufs=1))
        spool = ctx.enter_context(tc.tile_pool(name="spool", bufs=1))

        x_sb = resid.tile([128, NJ, HW], BF, tag="x")
        wmc = resid.tile([128, 64, 8], BF, tag="wmc")
        bfc = resid.tile([128, 1312], BF, tag="bfc")
        sm = resid.tile([128, 272], F32, tag="sm")
        wg1_sb = resid.tile([128, 8, 2048], BF, tag="wg1")
        wg2_sb = resid.tile([128, 4096], BF, tag="wg2")
        # transposed slabs 0..NT-1: xT[q, j, g, c] = x[128j+c, 128g+q]
        xT = resid.tile([128, NT, 32, 128], BF, tag="xT")

        # ---- DMA issue order: consts, x (slab-halves), weights by round
        nc.sync.dma_start(wmc[:], wmc_d[:])
        nc.sync.dma_start(bfc[:], bfc_d[:])
        nc.sync.dma_start(sm[:], sm_d[:])
        for j in range(NJ):
            for h in range(2):
                nc.sync.dma_start(
                    x_sb[:, j, 2048 * h:2048 * (h + 1)],
                    x_d[128 * j:128 * (j + 1), 2048 * h:2048 * (h + 1)],
                )
        for j in range(NJ):
            nc.sync.dma_start(wg1_sb[:, j, :], wg1_d[0, j])
        nc.sync.dma_start(wg2_sb[:], wg2_d[0])

        ident = bfc[:, 0:128]
        ident8 = bfc[0:8, 0:8]
        ones_col_bf = bfc[:, 128:129]
        ones8_bf = bfc[0:8, 128:129]
        one0 = bfc[0:1, 128:129]

        def eg(g):
            return bfc[0:8, 288 + 128 * g:288 + 128 * (g + 1)]

        onesf = sm[0:1, 128:256]

        def b1c(r, b):   # [128, 8] column block for branch b (0=mul,1=add)
            return sm[:, 16 * r + 8 * b:16 * r + 8 * b + 8]

        def gc(r, b):
            return sm[:, 32 + 16 * r + 8 * b:32 + 16 * r + 8 * b + 8]

        def bec(r, b):
            return sm[:, 64 + 16 * r + 8 * b:64 + 16 * r + 8 * b + 8]

        def b2c(r, b):
            return sm[:, 96 + 16 * r + 8 * b:96 + 16 * r + 8 * b + 8]

        cm256 = sm[0:1, 257:258]   # -1/256 (LN variance fold)

        early_ctx = tc.tile_pool(name="early", bufs=1)
        early = early_ctx.__enter__()
        attn_bc = early.tile([128, HW], BF, tag="attn_bc")
        scrD = early.tile([128, HW // 4], BF, tag="scrD")

        # =============== phase A: logits ride the x DMA ===============
        attn8 = spool.tile([8, 512], BF, tag="attn8")
        acc8 = spool.tile([8, 1], F32, tag="acc8")
        inv = spool.tile([1, 1], F32, tag="inv")
        inv_bc = spool.tile([128, 1], F32, tag="inv_bc")
        v0 = spool.tile([128, NJ], F32, tag="v0")
        v0g = spool.tile([128, NJ], BF, tag="v0g")

        with tc.tile_pool(name="psA", bufs=1,
                          space=bass.MemorySpace.PSUM) as psA:
            lg8 = psA.tile([8, 512], F32, tag="lg8")

            def pass1_mms(j, start=False):
                for g in range(8):
                    nc.tensor.matmul(
                        lg8[:],
                        wmc[:, 8 * j + g, :],
                        x_sb[:, j, 512 * g:512 * (g + 1)],
                        start=(start and g == 0),
                        stop=(j == NJ - 1 and g == 7),
                    )

            def transpose_slab(j):
                # 2 copies to scalar, 2 to DVE per slab
                for t in range(4):
                    px = psA.tile([128, 1024], BF, tag="xp",
                                  name=f"xp{t % 3}", bufs=3)
                    for u in range(8):
                        g8 = 8 * t + u
                        nc.tensor.transpose(
                            px[:, 128 * u:128 * (u + 1)],
                            x_sb[:, j, 128 * g8:128 * (g8 + 1)],
                            ident,
                        )
                    src = px[:].rearrange("p (u c) -> p u c", c=128)
                    dst = xT[:, j, 8 * t:8 * (t + 1), :]
                    if t % 2 == 0:
                        nc.scalar.copy(dst, src)
                    else:
                        nc.vector.tensor_copy(dst, src)

            # PE order: ride arrivals, keep the pipe full
            pass1_mms(0, start=True)
            transpose_slab(0)
            pass1_mms(1)
            transpose_slab(1)
            pass1_mms(2)
            transpose_slab(2)
            pass1_mms(3)
            transpose_slab(3)
            for j in range(4, NJ):
                pass1_mms(j)
            transpose_slab(4)
            transpose_slab(5)

            # |logits| < ~4: softmax without max subtraction
            nc.scalar.activation(attn8[:], lg8[:], AF.Exp, accum_out=acc8[:])

            acc8b = spool.tile([8, 1], BF, tag="acc8b")
            nc.vector.tensor_copy(acc8b[:], acc8[:])
            ps_s = psA.tile([1, 1], F32, tag="small", bufs=1)
            nc.tensor.matmul(ps_s[:], acc8b[:], ones8_bf)
            nc.vector.reciprocal(inv[:], ps_s[:])
            ps_ib = psA.tile([128, 1], F32, tag="small", bufs=1)
            nc.tensor.matmul(ps_ib[:], onesf, inv[:])
            nc.vector.tensor_copy(inv_bc[:], ps_ib[:])
            scbf = spool.tile([128, 1], BF, tag="invbcb")
            nc.vector.tensor_copy(scbf[:], inv_bc[:])

            # attn columns: attnT[p, k, g] = attn[512g + 128k + p]
            psAT = psA.tile([128, 4, 8], BF, tag="small", bufs=1)
            for k in range(4):
                nc.tensor.transpose(
                    psAT[:, k, :], attn8[0:8, 128 * k:128 * (k + 1)],
                    ident8,
                )
            attnT = spool.tile([128, 4, 8], BF, tag="attnT")
            nc.vector.tensor_copy(attnT[:], psAT[:])

            # attn broadcast halves for the DVE STT slabs
            for g in range(8):
                pb = psA.tile([128, 512], F32, tag="bcb",
                              name=f"bc{g % 2}")
                nc.tensor.matmul(pb[:], eg(g), attn8[:])
                dst = attn_bc[:, 512 * g:512 * (g + 1)]
                if g % 2 == 0:
                    nc.scalar.copy(dst, pb[:])
                else:
                    nc.vector.tensor_copy(dst, pb[:])

            # ---- pass2 ctx ----
            # PE: slabs 0-3 as one xT block, then slabs 4-5
            ctx_ps = psA.tile([1, 512], F32, tag="ctx")
            for m in range(32):
                nc.tensor.matmul(
                    ctx_ps[:],
                    attnT[:, m % 4, m // 4:m // 4 + 1],
                    xT[:, 0:4, m, :],
                    start=(m == 0), stop=(m == 31),
                )
            ctx_ps2 = psA.tile([1, 256], F32, tag="ctx2")
            for m in range(32):
                nc.tensor.matmul(
                    ctx_ps2[:],
                    attnT[:, m % 4, m // 4:m // 4 + 1],
                    xT[:, 4:6, m, :],
                    start=(m == 0), stop=(m == 31),
                )
            v0row = spool.tile([1, 768], BF, tag="v0row")
            nc.vector.tensor_copy(v0row[0:1, 0:512], ctx_ps[:])
            nc.vector.tensor_copy(v0row[0:1, 512:768], ctx_ps2[:])
            psV = psA.tile([128, 6], F32, tag="small", bufs=1)
            for q in range(6):
                nc.tensor.matmul(
                    psV[:, q:q + 1],
                    v0row[0:1, 128 * q:128 * (q + 1)],
                    one0,
                )
            nc.vector.tensor_scalar_mul(v0[:, 0:6], psV[:], inv_bc[:])

            # DVE: slabs 6-7 via STT quarters (inv folded into scbf)
            v0p = spool.tile([128, NJ, 4], F32, tag="v0p")
            for h in range(4):
                hs = slice(1024 * h, 1024 * (h + 1))
                for j in (6, 7):
                    nc.vector.scalar_tensor_tensor(
                        out=scrD[:], in0=x_sb[:, j, hs], scalar=scbf[:],
                        in1=attn_bc[:, hs], op0=ALU.mult, op1=ALU.mult,
                        accum_out=v0p[:, j, h:h + 1],
                    )
            nc.vector.reduce_sum(
                out=v0[:, 6:NJ], in_=v0p[:, 6:NJ, :], axis=AX.X,
            )

        nc.vector.tensor_copy(v0g[:], v0[:])

        # =============== gates ===============
        gates_ctx = tc.tile_pool(name="psg", bufs=1,
                                 space=bass.MemorySpace.PSUM)
        psg = gates_ctx.__enter__()

        vmuls = []
        vadds = []
        vm1 = spool.tile([128, NJ], BF, tag="vm1")
        va1 = spool.tile([128, NJ], BF, tag="va1")

        def gate_round(r):
            tag = f"r{r}"

            def stat(b, j):
                if r == 0:
                    return v0g[:, j:j + 1]
                return (vm1 if b == 0 else va1)[:, j:j + 1]

            # W1: mul branch stream, then add branch stream
            psW = [psg.tile([1, 512], F32, tag="w1p", name=f"w1p{k}", bufs=4)
                   for k in range(4)]
            hrows = {}
            for b in range(2):
                for j in range(NJ):
                    for p in range(2):
                        nc.tensor.matmul(
                            psW[2 * b + p][:], stat(b, j),
                            wg1_sb[:, j,
                                   1024 * b + 512 * p:1024 * b + 512 * (p + 1)],
                            start=(j == 0), stop=(j == NJ - 1),
                        )
                # h rows -> sbuf bf16 (overlaps the next matmul stream)
                hrow = spool.tile([1, 1024], BF, tag="rowbuf",
                                  name=f"hrow{tag}{b}")
                nc.scalar.copy(hrow[0:1, 0:512], psW[2 * b][:])
                nc.vector.tensor_copy(hrow[0:1, 512:1024], psW[2 * b + 1][:])
                hrows[b] = hrow
            if r == 0:
                # round-1 W1 weights chase round-0's consumption (WAR)
                for j in range(NJ):
                    nc.sync.dma_start(wg1_sb[:, j, :], wg1_d[1, j])

            # h rows -> columns, (b, l, t) layout: col 8b + k
            psT = psg.tile([128, 16], F32, tag="tp", name=f"tp{tag}", bufs=2)
            for b in range(2):
                for k in range(8):
                    nc.tensor.matmul(
                        psT[:, 8 * b + k:8 * b + k + 1],
                        hrows[b][0:1, 128 * k:128 * (k + 1)],
                        one0,
                    )

            # LayerNorm, both branches batched on DVE (Ln/Exp/Relu on
            # Scalar).  g pre-scaled by sqrt(P) on host; invsigma_noP =
            # exp(-0.5*ln(S2 - S1^2/P + P*EPS)); mu folded as S1/P.
            hn_g = spool.tile([128, 16], BF, tag=f"hnbf{tag}")
            ps_st = psg.tile([1, 32], F32, tag="tiny", bufs=2)
            ps_bc2 = psg.tile([128, 32], F32, tag="tp", name=f"tpb{tag}",
                              bufs=2)

            stcat = spool.tile([128, 32], BF, tag="stcat")
            nc.vector.tensor_add(stcat[:, 0:16], psT[:],
                                 sm[:, 16 * r:16 * r + 16])
            nc.vector.tensor_mul(stcat[:, 16:32], stcat[:, 0:16],
                                 stcat[:, 0:16])
            nc.tensor.matmul(ps_st[:], ones_col_bf, stcat[:])

            w8 = spool.tile([1, 16], F32, tag="w8")
            nc.vector.reduce_sum(
                out=w8[0:1, 0:16],
                in_=ps_st[0:1, 0:32].rearrange("p (g t) -> p g t", t=2),
                axis=AX.X,
            )
            # w8[0:8] = S1 per (b, l), w8[8:16] = S2 per (b, l)
            sq = spool.tile([1, 16], F32, tag="sq")
            nc.vector.tensor_mul(sq[0:1, 0:8], w8[0:1, 0:8], w8[0:1, 0:8])
            nc.vector.scalar_tensor_tensor(
                out=sq[0:1, 8:16], in0=sq[0:1, 0:8], scalar=cm256,
                in1=w8[0:1, 8:16], op0=ALU.mult, op1=ALU.add,
            )
            nc.vector.tensor_scalar_add(sq[0:1, 8:16], sq[0:1, 8:16],
                                        P * EPS)
            lnv = spool.tile([1, 16], F32, tag="lnv")
            nc.scalar.activation(lnv[0:1, 0:8], sq[0:1, 8:16], AF.Ln)
            nc.scalar.activation(lnv[0:1, 8:16], lnv[0:1, 0:8], AF.Exp,
                                 scale=-0.5)

            brow = spool.tile([1, 32], F32, tag="brow")
            bview = brow[0:1, 0:16].rearrange("p (g t) -> p t g", t=2)
            iview = brow[0:1, 16:32].rearrange("p (g t) -> p t g", t=2)
            for t in range(2):
                nc.vector.tensor_scalar_mul(bview[:, t, :], w8[0:1, 0:8],
                                            1.0 / P)
                nc.vector.tensor_copy(iview[:, t, :], lnv[0:1, 8:16])
            nc.tensor.matmul(ps_bc2[:], onesf, brow[:])

            hn = spool.tile([128, 16], F32, tag="hn")
            nc.vector.tensor_sub(hn[:], stcat[:, 0:16], ps_bc2[:, 0:16])
            nc.vector.tensor_mul(hn[:], hn[:], ps_bc2[:, 16:32])
            nc.vector.tensor_mul(hn[:], hn[:],
                                 sm[:, 32 + 16 * r:48 + 16 * r])
            nc.vector.tensor_add(hn[:], hn[:],
                                 sm[:, 64 + 16 * r:80 + 16 * r])
            nc.scalar.activation(hn_g[:], hn[:], AF.Relu)

            # W2: mul branch then add branch; z rows in psum (row 0)
            psZ = [psg.tile([1, 512], F32, tag="w1p", name=f"w1p{k}", bufs=4)
                   for k in range(4)]
            zrows = {}
            for b in range(2):
                for lv in range(4):
                    for t in range(2):
                        nc.tensor.matmul(
                            psZ[2 * b + lv // 2][
                                0:1, 256 * (lv % 2):256 * (lv % 2) + 256],
                            hn_g[:, 8 * b + 2 * lv + t:8 * b + 2 * lv + t + 1],
                            wg2_sb[:, 1024 * lv + 512 * t + 256 * b:
                                   1024 * lv + 512 * t + 256 * b + 256],
                            start=(t == 0), stop=(t == 1),
                        )
                zrow = spool.tile([1, 1024], BF, tag="rowbuf",
                                  name=f"zrow{tag}{b}")
                nc.scalar.copy(zrow[0:1, 0:512], psZ[2 * b][:])
                nc.vector.tensor_copy(zrow[0:1, 512:1024], psZ[2 * b + 1][:])
                zrows[b] = zrow
            if r == 0:
                nc.sync.dma_start(wg2_sb[:], wg2_d[1])

            psZT = psg.tile([128, 16], F32, tag="tp", name=f"tpz{tag}",
                            bufs=2)
            for b in range(2):
                for k in range(8):
                    nc.tensor.matmul(
                        psZT[:, 8 * b + k:8 * b + k + 1],
                        zrows[b][0:1, 128 * k:128 * (k + 1)],
                        one0,
                    )

            # z columns + b2; sigmoid(zm) = 1/(1+exp(-zm))
            vmul = spool.tile([128, 8], F32, tag=f"vm{tag}")
            vadd = spool.tile([128, 8], F32, tag=f"va{tag}")
            zcm = spool.tile([128, 8], F32, tag="zcm")
            e = spool.tile([128, 8], F32, tag="sge")
            nc.vector.tensor_add(zcm[:], psZT[:, 0:8], b2c(r, 0))
            nc.scalar.activation(e[:], zcm[:], AF.Exp, scale=-1.0)
            nc.vector.tensor_scalar_add(e[:], e[:], 1.0)
            nc.vector.reciprocal(vmul[:], e[:])
            nc.vector.tensor_add(vadd[:], psZT[:, 8:16], b2c(r, 1))
            vmuls.append(vmul)
            vadds.append(vadd)
            if r == 0:
                nc.vector.tensor_copy(vm1[:], vmul[:])
                nc.gpsimd.tensor_copy(va1[:], vadd[:])

        gate_round(0)
        gate_round(1)

        mm_f = spool.tile([128, NJ], F32, tag="mmf")
        nc.vector.tensor_add(mm_f[:], vmuls[0][:], vmuls[1][:])
        ma_f = spool.tile([128, NJ], F32, tag="maf")
        nc.gpsimd.tensor_add(ma_f[:], vadds[0][:], vadds[1][:])
        gates_ctx.__exit__(None, None, None)
        early_ctx.__exit__(None, None, None)

        # =============== pass 3: output ===============
        late_ctx = tc.tile_pool(name="late", bufs=1)
        late = late_ctx.__enter__()

        addsum = spool.tile([128, 2], F32, tag="addsum")
        nc.vector.reduce_sum(
            out=addsum[:],
            in_=ma_f[:].rearrange("p (l t) -> p t l", t=2),
            axis=AX.X,
        )
        # diag(mul) stationaries for the PE half (out rows 0:128)
        diags = []
        for lv in range(4):
            dt_ = late.tile([128, 128], BF, tag=f"diag{lv}", name=f"diag{lv}")
            nc.vector.tensor_scalar_mul(dt_[:], ident, mm_f[:, 2 * lv:2 * lv + 1])
            diags.append(dt_)

        with tc.tile_pool(name="ps3", bufs=1,
                          space=bass.MemorySpace.PSUM) as ps3:
            # PE half: l-outer accumulation into 8 chunk banks
            chunks = [ps3.tile([128, 512], F32, tag=f"big{n}", name=f"big{n}")
                      for n in range(8)]
            for lv in range(4):
                for n in range(8):
                    nc.tensor.matmul(
                        chunks[n][:],
                        diags[lv][:],
                        x_sb[:, 2 * lv, 512 * n:512 * (n + 1)],
                        start=(lv == 0), stop=(lv == 3),
                    )
            # Scalar/DVE half for out rows 128:256, Pool merges:
            #   a = x7*m7 + addsum (scalar), bq = x5*m5 + a (DVE STT)
            #   c = x3*m3 (scalar), d = x1*m1 + c (DVE STT)
            #   out = b + d (Pool TT)
            tpA = [late.tile([128, 1024], F32, tag=f"tA{k}", name=f"tA{k}")
                   for k in range(2)]
            tpB = [late.tile([128, 1024], F32, tag=f"tB{k}", name=f"tB{k}")
                   for k in range(2)]
            ob3 = [late.tile([128, 1024], F32, tag=f"ob{k}", name=f"ob{k}")
                   for k in range(2)]
            for blk in range(4):
                bs = slice(1024 * blk, 1024 * (blk + 1))
                a = tpA[blk % 2]
                c = tpB[blk % 2]
                o = ob3[blk % 2]
                nc.scalar.activation(
                    a[:], x_sb[:, 7, bs], AF.Identity,
                    bias=addsum[:, 1:2], scale=mm_f[:, 7:8],
                )
                nc.vector.scalar_tensor_tensor(
                    out=a[:], in0=x_sb[:, 5, bs], scalar=mm_f[:, 5:6],
                    in1=a[:], op0=ALU.mult, op1=ALU.add,
                )
                nc.scalar.activation(
                    c[:], x_sb[:, 3, bs], AF.Identity,
                    bias=0.0, scale=mm_f[:, 3:4],
                )
                nc.vector.scalar_tensor_tensor(
                    out=c[:], in0=x_sb[:, 1, bs], scalar=mm_f[:, 1:2],
                    in1=c[:], op0=ALU.mult, op1=ALU.add,
                )
                nc.gpsimd.tensor_add(o[:], a[:], c[:])
                nc.sync.dma_start(out_d[128:256, bs], o[:])

            # drain PE chunks with the bias fold, stream out
            stg = [late.tile([128, 512], F32, tag=f"stg{k}", name=f"stg{k}")
                   for k in range(2)]
            for n in range(8):
                s = stg[n % 2]
                if n % 2 == 0:
                    nc.scalar.activation(
                        s[:], chunks[n][:], AF.Identity,
                        bias=addsum[:, 0:1], scale=1.0,
                    )
                else:
                    nc.vector.tensor_scalar_add(s[:], chunks[n][:],
                                                addsum[:, 0:1])
                nc.sync.dma_start(
                    out_d[0:128, 512 * n:512 * (n + 1)], s[:],
                )
        late_ctx.__exit__(None, None, None)

    nc.compile()
    return nc


def _pack_inputs(x0, x1, x2, x3, wm, bm,
                 add_W1, add_b1, add_g, add_be, add_W2, add_b2,
                 mul_W1, mul_b1, mul_g, mul_be, mul_W2, mul_b2):
    bf = ml_dtypes.bfloat16
    f32 = np.float32

    # pass1 stationaries: [:, 8j+g, g] = wm slab j
    wmr = np.asarray(wm, f32).reshape(NJ, 128)
    wmc = np.zeros((128, 64, 8), f32)
    for j in range(NJ):
        for g in range(8):
            wmc[:, 8 * j + g, g] = wmr[j]
    wmc = wmc.astype(bf)

    # bf16 const block: identity | ones col | pad | eg selectors
    bfc = np.zeros((128, 1312), f32)
    bfc[:, 0:128] = np.eye(128)
    bfc[:, 128] = 1.0
    for g in range(8):
        bfc[g, 288 + 128 * g:288 + 128 * (g + 1)] = 1.0
    bfc = bfc.astype(bf)

    W1s = [[mul_W1[r], add_W1[r]] for r in range(R)]
    W2s = [[mul_W2[r], add_W2[r]] for r in range(R)]
    b1s = [[mul_b1[r], add_b1[r]] for r in range(R)]
    gs = [[mul_g[r], add_g[r]] for r in range(R)]
    bes = [[mul_be[r], add_be[r]] for r in range(R)]
    b2s = [[mul_b2[r], add_b2[r]] for r in range(R)]

    sm = np.zeros((128, 272), f32)
    sm[:, 128:256] = 1.0
    sm[:, 256] = 1.0 / 256.0
    sm[:, 257] = -1.0 / 256.0

    def colmajor(v):  # [4,256]-like -> [128, 8] cols (l, t)
        return np.asarray(v, f32).reshape(4, 2, 128).transpose(2, 0, 1).reshape(128, 8)

    wg1 = np.zeros((2, 8, 128, 2048), f32)
    wg2 = np.zeros((2, 128, 4096), f32)

    for r in range(R):
        w2arr = np.zeros((128, 4, 2, 2, 256), f32)   # [pp, l, t, b, cl]
        for b in range(2):
            w1 = np.asarray(W1s[r][b], f32).reshape(C, C)  # [lp, c]
            t = w1.reshape(C, NJ, 128)                   # [q, j, cp]
            t = t.transpose(1, 2, 0)                     # [j, cp, q]
            wg1[r, :, :, 1024 * b:1024 * (b + 1)] = t
            w2 = np.asarray(W2s[r][b], f32)              # [l, cl, pp]
            t2 = w2.reshape(4, 256, 2, 128)              # [l, cl, tt, pp]
            t2 = t2.transpose(3, 0, 2, 1)                # [pp, l, tt, cl]
            w2arr[:, :, :, b, :] = t2
            sm[:, 16 * r + 8 * b:16 * r + 8 * b + 8] = colmajor(b1s[r][b])
            sm[:, 32 + 16 * r + 8 * b:32 + 16 * r + 8 * b + 8] = \
                colmajor(gs[r][b]) * float(np.sqrt(P))
            sm[:, 64 + 16 * r + 8 * b:64 + 16 * r + 8 * b + 8] = \
                colmajor(bes[r][b])
            sm[:, 96 + 16 * r + 8 * b:96 + 16 * r + 8 * b + 8] = \
                colmajor(b2s[r][b])
        wg2[r] = w2arr.reshape(128, 4096)

    shared = dict(wmc=wmc, bfc=bfc, smalls=sm,
                  wg1=wg1.astype(bf), wg2=wg2.astype(bf))

    in_maps = []
    xs = [np.asarray(a, f32) for a in (x0, x1, x2, x3)]
    for b in range(B):
        xc = np.concatenate(
            [a[b].reshape(CL, HW) for a in xs], axis=0
        ).astype(bf)
        in_maps.append({"x": xc, **shared})
    return in_maps


def kernel(**inputs):
    from concourse.bass_utils import run_bass_kernel_spmd

    if "nc" not in _CACHE:
        _CACHE["nc"] = _build_nc()
    nc = _CACHE["nc"]

    in_maps = _pack_inputs(**inputs)
    res = run_bass_kernel_spmd(nc, in_maps, list(range(NCORES)))
    _CACHE["last_results"] = res
    out = np.stack(
        [res.results[b]["out"].reshape(CL, H, W) for b in range(B)]
    ).astype(np.float32)
    return out
